# revision 49
# baseline (speedup 1.0000x reference)
"""Trainium2 Bass kernel for grouped-correlation multi-view warping (MVS similarity).

Computation (original nn.Module): for each source view s, warp src_fea[s] to the
reference view at D depth hypotheses via per-pixel projection, then accumulate
grouped correlation with the reference feature:
    sim_sum[b,g,d,h,w] = sum_s mean_{c in g} warped[s,b,c,d,h,w] * ref[b,c,h,w]

Key structural property of this module's input distribution: the projection
chain composes INTR_INV twice, so for near-identity extrinsics every projected
point lands in the [0,1) x [0,1) pixel cell (or is masked out-of-bounds to
exactly (0,0)): the bilinear taps are always the four corner pixels
(0,0),(0,1),(1,0),(1,1), and only the bilinear WEIGHTS (fx=px, fy=py) vary per
output element.  The host verifies this cheaply for the actual inputs; if any
assumption fails we fall back to a general host-side computation.

With w0 := 1, w_{1..3} := (fx, fy, fx*fy) of view 0, w_{4..6} of view 1, and
DOT_k[g,hw] := (1/4) sum_{c in g} ref[c,hw] * combo_k[c] (combo = corner-tap
combinations), the output is the rank-7 contraction

    sim[g,d,hw] = sum_{k=0}^{6} DOT_k[g,hw] * W_k[d,hw].

Device mapping (per core = one (batch, depth-quarter), 12 planes):
  All on-chip tensors use pixel partitions p2 = (w%2)*64 + h%64 with
  free index (w//2, ...); the host pre-shuffles dep/rx and un-shuffles
  the output, which makes the DOT transpose a single DMA:
  - DOT build on the TENSOR engine: matmuls contracting channels,
    stationary = block-diagonal combo matrix [2*32, 2*56] (two h-halves
    packed into the contraction dim, output partitions ordered
    q=(k*8+g)*2+pp), moving = ref features [64, 10240] fp16 (pixel
    order (w, h64)) -> PSUM [112, 2048]-groups; scalar/DVE drain to
    SBUF fp16; ONE SBUF->SBUF hardware XBAR DMA-transpose
    (out[p,a,n] = in[n, a*128+p]) lands DOT directly in compute layout
    [p2, (w2, k, g, pp)] -- no DRAM bounce, no scatter DMAs.
  - Projection chain: X/Y/Z = rx*dep on GpSimd (fp16), Z bias on the
    scalar engine, 1/Z via DVE reciprocal_approx_fast (f32),
    fx = relu(X + t0) (scalar, fused bias+relu; valid because rZ > 0)
    times rZ (DVE fp16 2x).
  - Accumulation: DVE streams the 6 products tmp_k = DOT_k (x) W_k
    (fp16, 2x mode) per 4-plane chunk -- this stream is the critical
    path; the TENSOR engine absorbs the 7-term sum behind it with
    identity-stationary PSUM-accumulate matmuls emitted TERM-MAJOR
    (base DOT_0 closes each 512-col region), scalar engine drains PSUM
    -> fp16 staging; the last chunk leaves one plane on DVE in-place
    adds so the PE tail and DVE tail finish together; DMA writes fp16
    output (host converts to f32).

Sharding: 8 cores = 2 batches x 4 depth-quarters (12 planes each); outputs are
disjoint -> no collectives.
"""

import sys

sys.path.insert(0, "/opt/trn_rl_repo")

import numpy as np

B, C, H, W, D, S, G = 2, 32, 128, 160, 48, 2, 8
HW = H * W
CPG = C // G
NCORES = 8
DQ = D // 4  # depth planes per core (12)
DCH = 4  # planes per chunk
NCH = DQ // DCH  # chunks (3)
H2 = H // 2  # 64
KG = 7 * G  # 56
PW = H2 * W  # pixels per h-half = 10240

INTR = np.array(
    [[361.54126, 0.0, 102.9005], [0.0, 360.39624, 77.38375], [0.0, 0.0, 1.0]],
    np.float32,
)
INTR_INV = np.array(
    [[0.00276594, 0.0, -0.2846162], [0.0, 0.00277472, -0.21471854], [0.0, 0.0, 1.0]],
    np.float32,
)

_PROGRAM_CACHE = {}


def _build_program():
    if "nc" in _PROGRAM_CACHE:
        return _PROGRAM_CACHE["nc"]

    import concourse.bacc as bacc
    import concourse.mybir as mybir
    import concourse.tile as tile

    f32 = mybir.dt.float32
    f16 = mybir.dt.float16
    Alu = mybir.AluOpType
    Act = mybir.ActivationFunctionType

    nc = bacc.Bacc("TRN2", target_bir_lowering=False, debug=False)

    # ref features, h-half-packed: reft[pp*32+c, h64*W+w] = 0.25*ref[c, (pp*64+h64)*W+w]
    reft = nc.dram_tensor("reft", [2 * C, PW], f16, kind="ExternalInput")
    # block-diagonal combo stationary: [pp*32+c, pp*56 + k*8+g]
    combos = nc.dram_tensor("combos", [2 * C, 2 * KG], f16, kind="ExternalInput")
    ident = nc.dram_tensor("ident", [H, H], f16, kind="ExternalInput")
    # rx[h, (v*3+j)*W + w] fp16 rotation rows per view
    rxh = nc.dram_tensor("rxh", [H, S * 3 * W], f16, kind="ExternalInput")
    tvec = nc.dram_tensor("tvec", [H, 8], f32, kind="ExternalInput")
    # depth, h-partition layout: [h, d*W+w] fp16
    dep = nc.dram_tensor("dep", [H, DQ * W], f16, kind="ExternalInput")
    # out free layout per plane: (w2, g, pp) -- host unshuffles
    out = nc.dram_tensor("out", [DQ, H, G * W], f16, kind="ExternalOutput")
    # partition layout everywhere below: p2 = (w%2)*64 + h%64; pixel
    # (h, w) lives at partition p2, free index (w//2, h//64) -- chosen so
    # ONE SBUF->SBUF XBAR DMA-transpose of the DOT matmul result lands
    # directly in compute layout (no DRAM bounce needed)

    with tile.TileContext(nc) as tc:
        with (
            tc.tile_pool(name="static", bufs=1) as ps,
            tc.tile_pool(name="wpool", bufs=1) as pwt,
        ):
            # ---------------- input loads (sync DMA queue) --------
            ident_t = ps.tile([H, H], f16, tag="ident")
            rxh_t = ps.tile([H, S * 3 * W], f16, tag="rxh")
            nc.sync.dma_start(rxh_t[:], rxh[:])
            tvec_t = ps.tile([H, 8], f32, tag="tvec")
            nc.sync.dma_start(tvec_t[:], tvec[:])
            dep_t = ps.tile([H, DQ * W], f16, tag="dep")
            nc.sync.dma_start(dep_t[:], dep[:])
            # DOT in compute layout: [p2, w2*112 + k*16 + g*2 + pp] fp16
            dot_all = ps.tile([H, KG * W], f16, tag="dot_all")

            # ---------------- DOT build (tensor engine) ----------------
            # pixel order inside ref_t/dot_sb is (w, h64): pix2 = w*64+h64,
            # so each 2048-column group is a clean 32-wide w-range.
            NJJ = PW // 2048  # 5 pipelined groups
            with (
                tc.tile_pool(name="boot", bufs=NJJ) as pb,
                tc.tile_pool(name="scratch", bufs=2) as pc,
                tc.tile_pool(name="dotpsum", bufs=2, space="PSUM") as pdp,
            ):
                combos_t = pb.tile([2 * C, 2 * KG], f16, tag="combos", bufs=1)
                nc.sync.dma_start(combos_t[:], combos[:])
                nc.sync.dma_start(ident_t[:], ident[:])
                dot_sb = pb.tile([2 * KG, PW], f16, tag="dot_sb", bufs=1)
                for jj in range(NJJ):
                    sl = slice(jj * 2048, (jj + 1) * 2048)
                    reft_t = pb.tile([2 * C, 2048], f16, tag="reft")
                    nc.scalar.dma_start(reft_t[:], reft[:, sl])
                    pt = pdp.tile([2 * KG, 2048], f32, tag="dotp")
                    for j4 in range(4):
                        s0 = 512 * j4
                        nc.tensor.matmul(
                            pt[:, s0 : s0 + 512],
                            combos_t[:],
                            reft_t[:, s0 : s0 + 512],
                            start=True,
                            stop=True,
                        )
                    # drain in two half-column pieces on both engines so
                    # each group's drain latency halves
                    h0 = jj * 2048
                    nc.scalar.activation(
                        dot_sb[:, h0 : h0 + 1024], pt[:, :1024], Act.Copy
                    )
                    nc.vector.tensor_copy(
                        dot_sb[:, h0 + 1024 : h0 + 2048], pt[:, 1024:]
                    )

                # ONE SBUF->SBUF XBAR DMA transpose straight into compute
                # layout: out[p2, w2, q] = dot_sb[q, w2*128 + p2]
                nc.sync.dma_start(
                    dot_all[:].rearrange("p (a n) -> p a n", a=W // 2),
                    dot_sb[:],
                    transpose=True,
                )

                # ------------ projection chain (both views) ------------
                # (emitted here but runs on gpsimd/scalar/DVE, overlapping
                # the PE DOT build; scratch tiles are tag-shared across
                # views)
                wts = {}
                for v in range(S):
                    rx = [
                        rxh_t[:, (v * 3 + j) * W : (v * 3 + j + 1) * W]
                        .rearrange("p (w2 pp) -> p w2 pp", pp=2)
                        .unsqueeze(1)
                        .to_broadcast([H, DQ, W // 2, 2])
                        for j in range(3)
                    ]
                    tb = [tvec_t[:, v * 3 + j : v * 3 + j + 1] for j in range(3)]
                    dsl = dep_t[:].rearrange(
                        "p (d w2 pp) -> p d w2 pp", d=DQ, pp=2
                    )

                    Xt = pc.tile([H, DQ * W], f16, tag="X", name=f"X{v}")
                    Yt = pc.tile([H, DQ * W], f16, tag="Y", name=f"Y{v}")
                    Zt = pc.tile([H, DQ * W], f32, tag="Z", name=f"Z{v}")
                    X = Xt[:].rearrange("p (d w2 pp) -> p d w2 pp", d=DQ, pp=2)
                    Y = Yt[:].rearrange("p (d w2 pp) -> p d w2 pp", d=DQ, pp=2)
                    Z = Zt[:].rearrange("p (d w2 pp) -> p d w2 pp", d=DQ, pp=2)
                    # X,Y on gpsimd (fp16); Z on gpsimd, f32 out for recip
                    nc.gpsimd.tensor_tensor(X, rx[0], dsl, Alu.mult)
                    nc.gpsimd.tensor_tensor(Y, rx[1], dsl, Alu.mult)
                    nc.gpsimd.tensor_tensor(Z, rx[2], dsl, Alu.mult)
                    # Z += t2 (f32, in place), rZ = 1/Z (f32), rZh = fp16(rZ)
                    nc.scalar.activation(
                        Zt[:], Zt[:], Act.Identity, bias=tb[2], scale=1.0
                    )
                    rZ = pc.tile([H, DQ * W], f32, tag="rZ", name=f"rZ{v}")
                    nc.vector.reciprocal_approx_fast(rZ[:], Zt[:])
                    rZh = pc.tile([H, DQ * W], f16, tag="rZh", name=f"rZh{v}")
                    nc.scalar.activation(rZh[:], rZ[:], Act.Copy)
                    # X = relu(X + t0) in place (relu commutes with *rZ>0)
                    nc.scalar.activation(
                        Xt[:], Xt[:], Act.Relu, bias=tb[0], scale=1.0
                    )
                    nc.scalar.activation(
                        Yt[:], Yt[:], Act.Relu, bias=tb[1], scale=1.0
                    )
                    # fx = Xr*rZ, fy = Yr*rZ, ff = fx*fy (DVE fp16 2x)
                    fx = pwt.tile([H, DQ * W], f16, tag=f"fx{v}", name=f"fx{v}")
                    fy = pwt.tile([H, DQ * W], f16, tag=f"fy{v}", name=f"fy{v}")
                    ff = pwt.tile([H, DQ * W], f16, tag=f"ff{v}", name=f"ff{v}")
                    nc.vector.tensor_tensor(fx[:], Xt[:], rZh[:], Alu.mult)
                    nc.vector.tensor_tensor(fy[:], Yt[:], rZh[:], Alu.mult)
                    nc.vector.tensor_tensor(ff[:], fx[:], fy[:], Alu.mult)
                    wts[v] = (fx, fy, ff)

            # ------------ accumulation (DVE products + PE matmul sum) ----
            GW = G * W  # 1280 columns per depth plane
            # per-plane matmul column segments (<=512)
            segs = [(0, 512), (512, 1024), (1024, 1280)]

            W2 = W // 2
            dview = dot_all[:].rearrange(
                "p (w2 k g pp) -> p w2 k g pp", k=7, g=G, pp=2
            )

            def dotk(k):
                # DOT_k[p2, (w2, g, pp)] broadcast over DCH depth planes
                return (
                    dview[:, :, k, :, :]
                    .unsqueeze(1)
                    .to_broadcast([H, DCH, W2, G, 2])
                )

            # planes handled by the tensor engine per chunk (rest go to DVE
            # in-place fp16 adds).  All planes on PE: the DVE product stream
            # is the critical path, and the ramped-up PE absorbs the whole
            # k-accumulation behind it.
            PE_PLANES = (4, 4, 3)
            # products per chunk offloaded to the gpsimd engine (0: the Q7
            # software tensor_tensor is far below DVE rate and contends for
            # SBUF ports with concurrent DVE passes)
            POOL_PRODS = (0, 0, 0)

            with (
                tc.tile_pool(name="prod", bufs=12) as pp,
                tc.tile_pool(name="ostage", bufs=2) as po,
                tc.tile_pool(name="mmpsum", bufs=2, space="PSUM") as pmm,
            ):
                for ch in range(NCH):
                    npe = PE_PLANES[ch]
                    tmps = []
                    kws = []
                    for v in range(S):
                        for wi, k in zip(wts[v], (1 + 3 * v, 2 + 3 * v, 3 + 3 * v)):
                            kws.append((wi, k))
                    # gpsimd-offloaded products first (emitted early so the
                    # pool engine starts while DVE streams its own products)
                    npool = POOL_PRODS[ch]
                    order = kws[:npool] + kws[npool:]
                    for i, (wi, k) in enumerate(order):
                        wv = (
                            wi[:]
                            .rearrange("p (d w2 pp) -> p d w2 pp", d=DQ, pp=2)[
                                :, ch * DCH : (ch + 1) * DCH, :, :
                            ]
                            .unsqueeze(3)
                            .to_broadcast([H, DCH, W2, G, 2])
                        )
                        tm = pp.tile([H, DCH * GW], f16, tag="tmp", name=f"tm{ch}")
                        tv_ = tm[:].rearrange(
                            "p (d w2 g pp) -> p d w2 g pp", d=DCH, g=G, pp=2
                        )
                        eng = nc.gpsimd if i < npool else nc.vector
                        eng.tensor_tensor(tv_, dotk(k), wv, Alu.mult)
                        tmps.append((tm, k))

                    # consume the gpsimd product(s) last so DVE/PE don't
                    # stall waiting for the slower pool engine
                    tmps_l = tmps[npool:] + tmps[:npool]

                    ost = po.tile([H, DCH * GW], f16, tag="ost", name=f"ost{ch}")
                    # --- PE-owned planes: PSUM accumulate + scalar drain ---
                    # TERM-MAJOR matmul emission within each plane: the base
                    # (DOT_0, available immediately) OPENS every region, each
                    # term streams as soon as its product lands, and the last
                    # product closes -- so only ~one term of matmuls remains
                    # after the final product.
                    for d in range(npe):
                        pt = pmm.tile([H, GW], f32, tag="accp")
                        for s0, s1 in segs:
                            nc.tensor.matmul(
                                pt[:, s0:s1],
                                ident_t[:],
                                dview[:, s0 // 16 : s1 // 16, 0, :, :],
                                start=True,
                                stop=False,
                            )
                        for i, (tm, k) in enumerate(tmps_l):
                            for s0, s1 in segs:
                                nc.tensor.matmul(
                                    pt[:, s0:s1],
                                    ident_t[:],
                                    tm[:, d * GW + s0 : d * GW + s1],
                                    start=False,
                                    stop=(i == len(tmps_l) - 1),
                                )
                        nc.scalar.activation(
                            ost[:, d * GW : (d + 1) * GW], pt[:], Act.Copy
                        )

                    # --- DVE-owned planes: fp16 in-place adds ---
                    if npe < DCH:
                        nd = DCH - npe
                        osl = ost[:, npe * GW : DCH * GW]
                        base = (
                            dview[:, :, 0, :, :]
                            .unsqueeze(1)
                            .to_broadcast([H, nd, W2, G, 2])
                        )
                        ov = osl.rearrange(
                            "p (d w2 g pp) -> p d w2 g pp", d=nd, g=G, pp=2
                        )
                        tm0, _ = tmps_l[0]
                        nc.vector.tensor_tensor(
                            ov, base, tm0[:, npe * GW :].rearrange(
                                "p (d w2 g pp) -> p d w2 g pp", d=nd, g=G, pp=2
                            ),
                            Alu.add,
                        )
                        for tm, k in tmps_l[1:]:
                            nc.vector.tensor_tensor(
                                osl, osl, tm[:, npe * GW :], Alu.add
                            )

                    nc.sync.dma_start(
                        out[ch * DCH : (ch + 1) * DCH, :, :].rearrange(
                            "d p c -> p d c"
                        ),
                        ost[:].rearrange("p (d c) -> p d c", d=DCH),
                    )

    nc.compile()
    _PROGRAM_CACHE["nc"] = nc
    return nc


def _host_prep(ref_feature, src_features, ref_proj, src_projs, depth_sample):
    """Projection-matrix chain bit-matched to the reference via jax CPU."""
    import jax
    import jax.numpy as jnp

    rot_xyz_all = np.zeros((S, B, 3, H, W), np.float32)
    trans_all = np.zeros((S, B, 3), np.float32)
    with jax.default_device(jax.devices("cpu")[0]):
        intr = jnp.asarray(INTR)
        intr_inv = jnp.asarray(INTR_INV)
        ref_p = intr_inv @ jnp.asarray(np.asarray(ref_proj))[:, :3, :4]  # [B,3,4]
        yy, xx = jnp.meshgrid(
            jnp.arange(H, dtype=jnp.float32), jnp.arange(W, dtype=jnp.float32),
            indexing="ij",
        )
        xyz = jnp.stack([xx.ravel(), yy.ravel(), jnp.ones(H * W, jnp.float32)])
        for s in range(S):
            src_p = intr_inv @ jnp.asarray(np.asarray(src_projs)[s])[:, :3, :4]
            proj = jnp.einsum("bij,bkj->bik", src_p[:, :, :3], ref_p[:, :, :3])
            trans = intr @ (src_p[:, :, 3:4] - proj @ ref_p[:, :, 3:4])
            rot = intr @ proj @ intr_inv
            rot_xyz = rot @ xyz  # [B,3,HW]
            rot_xyz_all[s] = np.asarray(rot_xyz).reshape(B, 3, H, W)
            trans_all[s] = np.asarray(trans).reshape(B, 3)

    # tap vectors: the 2x2 corner footprint of each (s,b) source image
    feats = np.asarray(src_features)
    tapv = np.zeros((S, B, 4, C), np.float32)
    for ti, (ty, tx) in enumerate(((0, 0), (0, 1), (1, 0), (1, 1))):
        tapv[:, :, ti, :] = feats[:, :, :, ty, tx]

    return rot_xyz_all, trans_all, tapv


def _check_degenerate(rot_xyz, trans, dep):
    """Verify, in a float32 mirror of the device computation, that for every
    pixel/plane/view: Z > 0.001 (zpos never fires), px,py < 1 (floor == 0 and
    the upper in-bounds masks never fire).  px,py >= 0 is NOT required (the
    device applies the >=0 mask via relu).  Conservative margins cover the
    device's fp16/approx-reciprocal differences."""
    for s in range(S):
        for b in range(B):
            rx = rot_xyz[s, b]
            t = trans[s, b]
            dq = dep[b]
            Z = rx[2] * dq + t[2]
            if Z.min() <= 0.0011:
                return False
            for k in (0, 1):
                P = (rx[k] * dq + t[k]) / Z
                if P.max() >= 0.995:
                    return False
    return True


def _fallback_numpy(rot_xyz, trans, refb, dep, src_features):
    """General (gather-based) host computation, used only if the degenerate
    fast-path assumption fails for the given inputs."""
    feats = np.asarray(src_features)
    P = np.ascontiguousarray(feats.transpose(0, 1, 3, 4, 2))  # [S,B,H,W,C]
    Px = np.roll(P, -1, axis=3)
    Py = np.roll(P, -1, axis=2)
    Pxy = np.roll(Py, -1, axis=3)
    tabs = np.concatenate([P, Px, Py, Pxy], axis=-1).reshape(S, B, HW, 4 * C)
    full = np.zeros((B, G, D, H, W), np.float32)
    for b in range(B):
        refb_b = refb[b].reshape(H, W, C)
        simacc = np.zeros((D, H, W, G), np.float32)
        for v in range(S):
            rx = rot_xyz[v, b][:, None]
            t = trans[v, b]
            dq = dep[b]
            X = rx[0] * dq + t[0]
            Y = rx[1] * dq + t[1]
            Z = rx[2] * dq + t[2]
            zm = (Z > 0.001).astype(np.float32)
            X, Y = X * zm, Y * zm
            Zc = np.where(Z > 0.001, Z, np.float32(1.0))
            px = X / Zc
            py = Y / Zc
            px = px * ((px < W) & (px >= 0)).astype(np.float32)
            py = py * ((py < H) & (py >= 0)).astype(np.float32)
            fx = px - np.floor(px)
            fy = py - np.floor(py)
            x0 = px - fx
            y0 = py - fy
            gx = np.float32(1.0) - fx
            gy = np.float32(1.0) - fy
            wts = [gx * gy, fx * gy, gx * fy, fx * fy]
            idx = (y0 * W + x0).astype(np.int32)
            gat = tabs[v, b][idx]
            R = (
                gat.reshape(D, H, W, 4, G, CPG)
                * refb_b.reshape(1, H, W, 1, G, CPG)
            ).sum(axis=-1)
            simacc += sum(R[:, :, :, ti, :] * wts[ti][..., None] for ti in range(4))
        full[b] = simacc.transpose(3, 0, 1, 2)
    return full


def _make_in_maps(ref_feature, src_features, ref_proj, src_projs, depth_sample):
    rot_xyz, trans, tapv = _host_prep(
        ref_feature, src_features, ref_proj, src_projs, depth_sample
    )
    dep = np.asarray(depth_sample)
    if not _check_degenerate(rot_xyz, trans, dep):
        refb = (
            np.asarray(ref_feature).transpose(0, 2, 3, 1) * np.float32(0.25)
        ).reshape(B, H, W * C)
        return None, (rot_xyz, trans, refb, dep)

    ref = np.asarray(ref_feature)  # [B,C,H,W]
    ident = np.eye(H, dtype=np.float16)

    # per-batch tensors
    reft_b = {}
    combos_b = {}
    rxh_b = {}
    tvec_b = {}
    for b in range(B):
        # reft[pp*32+c, w*64+h64] = ref[c, pp*64+h64, w]  (0.25 baked into combos)
        rt = (
            ref[b].reshape(C, 2, H2, W).transpose(1, 0, 3, 2).reshape(2 * C, PW)
        )
        reft_b[b] = rt.astype(np.float16)

        # combos (0.25-scaled), block-diag over pp and group-diag over g;
        # output-partition order q = (k*8+g)*2 + pp so the XBAR transpose
        # lands DOT in (w2, k, g, pp) free order:
        # combos[pp*32+c, (k*8+g)*2+pp] = 0.25*combo_k[c] if c//4==g
        A0, B0, C0, D0 = tapv[0, b]
        A1, B1, C1, D1 = tapv[1, b]
        ck = np.stack(
            [A0 + A1, B0 - A0, C0 - A0, A0 - B0 - C0 + D0,
             B1 - A1, C1 - A1, A1 - B1 - C1 + D1]
        ) * np.float32(0.25)  # [7, C]
        cb = np.zeros((2 * C, 2 * KG), np.float32)
        for k in range(7):
            for c in range(C):
                g = c // CPG
                for pps in range(2):
                    cb[pps * C + c, (k * G + g) * 2 + pps] = ck[k, c]
        combos_b[b] = cb.astype(np.float16)

        rx = rot_xyz[:, b]  # [S,3,H,W]
        # [p2=(wpar,h64), (j, w2, pp)]
        rxh_b[b] = (
            rx.reshape(S * 3, 2, H2, W // 2, 2)
            .transpose(4, 2, 0, 3, 1)
            .reshape(H, S * 3 * W)
        ).astype(np.float16)
        tv = np.zeros((H, 8), np.float32)
        tv[:, 0:3] = trans[0, b]
        tv[:, 3:6] = trans[1, b]
        tvec_b[b] = tv

    in_maps = []
    for kcore in range(NCORES):
        b, q = kcore // 4, kcore % 4
        dslc = dep[b, q * DQ : (q + 1) * DQ]  # [DQ,H,W]
        # [p2=(wpar,h64), (d, w2, pp)]
        dep_hp = (
            dslc.reshape(DQ, 2, H2, W // 2, 2)
            .transpose(4, 2, 0, 3, 1)
            .reshape(H, DQ * W)
            .astype(np.float16)
        )
        in_maps.append(
            {
                "reft": reft_b[b],
                "combos": combos_b[b],
                "ident": ident,
                "rxh": rxh_b[b],
                "tvec": tvec_b[b],
                "dep": np.ascontiguousarray(dep_hp),
            }
        )
    return in_maps, None


def kernel(ref_feature, src_features, ref_proj, src_projs, depth_sample):
    from concourse.bass_utils import run_bass_kernel_spmd

    in_maps, fb = _make_in_maps(
        ref_feature, src_features, ref_proj, src_projs, depth_sample
    )
    if in_maps is None:
        rot_xyz, trans, refb, dep = fb
        return _fallback_numpy(rot_xyz, trans, refb, dep, src_features)

    nc = _build_program()
    res = run_bass_kernel_spmd(nc, in_maps, core_ids=list(range(NCORES)))

    full = np.zeros((B, G, D, H, W), np.float32)
    for kcore in range(NCORES):
        b, q = kcore // 4, kcore % 4
        # out[d, p2=(wpar,h64), (w2, g, pp)] -> [g, d, h=(pp,h64), w=(w2,wpar)]
        o = res.results[kcore]["out"].astype(np.float32)
        o = o.reshape(DQ, 2, H2, W // 2, G, 2).transpose(4, 0, 5, 2, 3, 1)
        full[b, :, q * DQ : (q + 1) * DQ] = o.reshape(G, DQ, H, W)
    return full


# revision 50
# speedup vs baseline: 1.0292x; 1.0292x over previous
"""Trainium2 Bass kernel for grouped-correlation multi-view warping (MVS similarity).

Computation (original nn.Module): for each source view s, warp src_fea[s] to the
reference view at D depth hypotheses via per-pixel projection, then accumulate
grouped correlation with the reference feature:
    sim_sum[b,g,d,h,w] = sum_s mean_{c in g} warped[s,b,c,d,h,w] * ref[b,c,h,w]

Key structural property of this module's input distribution: the projection
chain composes INTR_INV twice, so for near-identity extrinsics every projected
point lands in the [0,1) x [0,1) pixel cell (or is masked out-of-bounds to
exactly (0,0)): the bilinear taps are always the four corner pixels
(0,0),(0,1),(1,0),(1,1), and only the bilinear WEIGHTS (fx=px, fy=py) vary per
output element.  The host verifies this cheaply for the actual inputs; if any
assumption fails we fall back to a general host-side computation.

With w0 := 1, w_{1..3} := (fx, fy, fx*fy) of view 0, w_{4..6} of view 1, and
DOT_k[g,hw] := (1/4) sum_{c in g} ref[c,hw] * combo_k[c] (combo = corner-tap
combinations), the output is the rank-7 contraction

    sim[g,d,hw] = sum_{k=0}^{6} DOT_k[g,hw] * W_k[d,hw].

Device mapping (per core = one (batch, depth-quarter), 12 planes):
  All on-chip tensors use pixel partitions p2 = (w%2)*64 + h%64 with
  free index (w//2, ...); the host pre-shuffles dep/rx and un-shuffles
  the output, which makes the DOT transpose a single DMA:
  - DOT build on the TENSOR engine: matmuls contracting channels,
    stationary = block-diagonal combo matrix [2*32, 2*56] (two h-halves
    packed into the contraction dim, output partitions ordered
    q=(k*8+g)*2+pp), moving = ref features [64, 10240] fp16 (pixel
    order (w, h64)) -> PSUM [112, 2048]-groups; scalar/DVE drain to
    SBUF fp16; ONE SBUF->SBUF hardware XBAR DMA-transpose
    (out[p,a,n] = in[n, a*128+p]) lands DOT directly in compute layout
    [p2, (w2, k, g, pp)] -- no DRAM bounce, no scatter DMAs.
  - Projection chain: X/Y/Z = rx*dep on GpSimd (fp16), Z bias on the
    scalar engine, 1/Z via DVE reciprocal_approx_fast (f32),
    fx = relu(X + t0) (scalar, fused bias+relu; valid because rZ > 0)
    times rZ (DVE fp16 2x).
  - Accumulation: DVE streams the 6 products tmp_k = DOT_k (x) W_k
    (fp16, 2x mode) per 4-plane chunk -- this stream is the critical
    path; the TENSOR engine absorbs the 7-term sum behind it with
    identity-stationary PSUM-accumulate matmuls emitted TERM-MAJOR
    (base DOT_0 closes each 512-col region), scalar engine drains PSUM
    -> fp16 staging; the last chunk leaves one plane on DVE in-place
    adds so the PE tail and DVE tail finish together; DMA writes fp16
    output (host converts to f32).

Sharding: 8 cores = 2 batches x 4 depth-quarters (12 planes each); outputs are
disjoint -> no collectives.
"""

import sys

sys.path.insert(0, "/opt/trn_rl_repo")

import numpy as np

B, C, H, W, D, S, G = 2, 32, 128, 160, 48, 2, 8
HW = H * W
CPG = C // G
NCORES = 8
DQ = D // 4  # depth planes per core (12)
DCH = 4  # planes per chunk
NCH = DQ // DCH  # chunks (3)
H2 = H // 2  # 64
KG = 7 * G  # 56
PW = H2 * W  # pixels per h-half = 10240

INTR = np.array(
    [[361.54126, 0.0, 102.9005], [0.0, 360.39624, 77.38375], [0.0, 0.0, 1.0]],
    np.float32,
)
INTR_INV = np.array(
    [[0.00276594, 0.0, -0.2846162], [0.0, 0.00277472, -0.21471854], [0.0, 0.0, 1.0]],
    np.float32,
)

_PROGRAM_CACHE = {}


def _build_program():
    if "nc" in _PROGRAM_CACHE:
        return _PROGRAM_CACHE["nc"]

    import concourse.bacc as bacc
    import concourse.mybir as mybir
    import concourse.tile as tile

    f32 = mybir.dt.float32
    f16 = mybir.dt.float16
    Alu = mybir.AluOpType
    Act = mybir.ActivationFunctionType

    nc = bacc.Bacc("TRN2", target_bir_lowering=False, debug=False)

    # ref features, h-half-packed: reft[pp*32+c, h64*W+w] = 0.25*ref[c, (pp*64+h64)*W+w]
    reft = nc.dram_tensor("reft", [2 * C, PW], f16, kind="ExternalInput")
    # block-diagonal combo stationary: [pp*32+c, pp*56 + k*8+g]
    combos = nc.dram_tensor("combos", [2 * C, 2 * KG], f16, kind="ExternalInput")
    ident = nc.dram_tensor("ident", [H, H], f16, kind="ExternalInput")
    # rx[h, (v*3+j)*W + w] fp16 rotation rows per view
    rxh = nc.dram_tensor("rxh", [H, S * 3 * W], f16, kind="ExternalInput")
    tvec = nc.dram_tensor("tvec", [H, 8], f32, kind="ExternalInput")
    # depth, h-partition layout: [h, d*W+w] fp16
    dep = nc.dram_tensor("dep", [H, DQ * W], f16, kind="ExternalInput")
    # out free layout per plane: (w2, g, pp) -- host unshuffles
    out = nc.dram_tensor("out", [DQ, H, G * W], f16, kind="ExternalOutput")
    # partition layout everywhere below: p2 = (w%2)*64 + h%64; pixel
    # (h, w) lives at partition p2, free index (w//2, h//64) -- chosen so
    # ONE SBUF->SBUF XBAR DMA-transpose of the DOT matmul result lands
    # directly in compute layout (no DRAM bounce needed)

    with tile.TileContext(nc) as tc:
        with (
            tc.tile_pool(name="static", bufs=1) as ps,
            tc.tile_pool(name="wpool", bufs=1) as pwt,
        ):
            # ---------------- input loads (sync DMA queue) --------
            ident_t = ps.tile([H, H], f16, tag="ident")
            rxh_t = ps.tile([H, S * 3 * W], f16, tag="rxh")
            nc.sync.dma_start(rxh_t[:], rxh[:])
            tvec_t = ps.tile([H, 8], f32, tag="tvec")
            nc.sync.dma_start(tvec_t[:], tvec[:])
            dep_t = ps.tile([H, DQ * W], f16, tag="dep")
            nc.sync.dma_start(dep_t[:], dep[:])
            # DOT in compute layout: [p2, w2*112 + k*16 + g*2 + pp] fp16
            dot_all = ps.tile([H, KG * W], f16, tag="dot_all")

            # ---------------- DOT build (tensor engine) ----------------
            # pixel order inside ref_t/dot_sb is (w, h64): pix2 = w*64+h64,
            # so each 2048-column group is a clean 32-wide w-range.
            NJJ = PW // 2048  # 5 pipelined groups
            with (
                tc.tile_pool(name="boot", bufs=NJJ) as pb,
                tc.tile_pool(name="scratch", bufs=2) as pc,
                tc.tile_pool(name="dotpsum", bufs=2, space="PSUM") as pdp,
            ):
                combos_t = pb.tile([2 * C, 2 * KG], f16, tag="combos", bufs=1)
                nc.sync.dma_start(combos_t[:], combos[:])
                nc.sync.dma_start(ident_t[:], ident[:])
                dot_sb = pb.tile([2 * KG, PW], f16, tag="dot_sb", bufs=1)
                for jj in range(NJJ):
                    sl = slice(jj * 2048, (jj + 1) * 2048)
                    reft_t = pb.tile([2 * C, 2048], f16, tag="reft")
                    nc.scalar.dma_start(reft_t[:], reft[:, sl])
                    pt = pdp.tile([2 * KG, 2048], f32, tag="dotp")
                    for j4 in range(4):
                        s0 = 512 * j4
                        nc.tensor.matmul(
                            pt[:, s0 : s0 + 512],
                            combos_t[:],
                            reft_t[:, s0 : s0 + 512],
                            start=True,
                            stop=True,
                        )
                    if jj % 2 == 0:
                        nc.scalar.activation(dot_sb[:, sl], pt[:], Act.Copy)
                    else:
                        nc.vector.tensor_copy(dot_sb[:, sl], pt[:])

                # ONE SBUF->SBUF XBAR DMA transpose straight into compute
                # layout: out[p2, w2, q] = dot_sb[q, w2*128 + p2]
                nc.sync.dma_start(
                    dot_all[:].rearrange("p (a n) -> p a n", a=W // 2),
                    dot_sb[:],
                    transpose=True,
                )

                # ------------ projection chain (both views) ------------
                # (emitted here but runs on gpsimd/scalar/DVE, overlapping
                # the PE DOT build; scratch tiles are tag-shared across
                # views)
                wts = {}
                for v in range(S):
                    rx = [
                        rxh_t[:, (v * 3 + j) * W : (v * 3 + j + 1) * W]
                        .rearrange("p (w2 pp) -> p w2 pp", pp=2)
                        .unsqueeze(1)
                        .to_broadcast([H, DQ, W // 2, 2])
                        for j in range(3)
                    ]
                    tb = [tvec_t[:, v * 3 + j : v * 3 + j + 1] for j in range(3)]
                    dsl = dep_t[:].rearrange(
                        "p (d w2 pp) -> p d w2 pp", d=DQ, pp=2
                    )

                    Xt = pc.tile([H, DQ * W], f16, tag="X", name=f"X{v}")
                    Yt = pc.tile([H, DQ * W], f16, tag="Y", name=f"Y{v}")
                    Zt = pc.tile([H, DQ * W], f32, tag="Z", name=f"Z{v}")
                    X = Xt[:].rearrange("p (d w2 pp) -> p d w2 pp", d=DQ, pp=2)
                    Y = Yt[:].rearrange("p (d w2 pp) -> p d w2 pp", d=DQ, pp=2)
                    Z = Zt[:].rearrange("p (d w2 pp) -> p d w2 pp", d=DQ, pp=2)
                    # X,Y on gpsimd (fp16); Z on gpsimd, f32 out for recip
                    nc.gpsimd.tensor_tensor(X, rx[0], dsl, Alu.mult)
                    nc.gpsimd.tensor_tensor(Y, rx[1], dsl, Alu.mult)
                    nc.gpsimd.tensor_tensor(Z, rx[2], dsl, Alu.mult)
                    # Z += t2 (f32, in place), rZ = 1/Z (f32), rZh = fp16(rZ)
                    nc.scalar.activation(
                        Zt[:], Zt[:], Act.Identity, bias=tb[2], scale=1.0
                    )
                    rZ = pc.tile([H, DQ * W], f32, tag="rZ", name=f"rZ{v}")
                    nc.vector.reciprocal_approx_fast(rZ[:], Zt[:])
                    rZh = pc.tile([H, DQ * W], f16, tag="rZh", name=f"rZh{v}")
                    nc.scalar.activation(rZh[:], rZ[:], Act.Copy)
                    # X = relu(X + t0) in place (relu commutes with *rZ>0)
                    nc.scalar.activation(
                        Xt[:], Xt[:], Act.Relu, bias=tb[0], scale=1.0
                    )
                    nc.scalar.activation(
                        Yt[:], Yt[:], Act.Relu, bias=tb[1], scale=1.0
                    )
                    # fx = Xr*rZ, fy = Yr*rZ, ff = fx*fy (DVE fp16 2x)
                    fx = pwt.tile([H, DQ * W], f16, tag=f"fx{v}", name=f"fx{v}")
                    fy = pwt.tile([H, DQ * W], f16, tag=f"fy{v}", name=f"fy{v}")
                    ff = pwt.tile([H, DQ * W], f16, tag=f"ff{v}", name=f"ff{v}")
                    nc.vector.tensor_tensor(fx[:], Xt[:], rZh[:], Alu.mult)
                    nc.vector.tensor_tensor(fy[:], Yt[:], rZh[:], Alu.mult)
                    nc.vector.tensor_tensor(ff[:], fx[:], fy[:], Alu.mult)
                    wts[v] = (fx, fy, ff)

            # ------------ accumulation (DVE products + PE matmul sum) ----
            GW = G * W  # 1280 columns per depth plane
            # per-plane matmul column segments (<=512)
            segs = [(0, 512), (512, 1024), (1024, 1280)]

            W2 = W // 2
            dview = dot_all[:].rearrange(
                "p (w2 k g pp) -> p w2 k g pp", k=7, g=G, pp=2
            )

            def dotk(k):
                # DOT_k[p2, (w2, g, pp)] broadcast over DCH depth planes
                return (
                    dview[:, :, k, :, :]
                    .unsqueeze(1)
                    .to_broadcast([H, DCH, W2, G, 2])
                )

            # planes handled by the tensor engine per chunk (rest go to DVE
            # in-place fp16 adds).  All planes on PE: the DVE product stream
            # is the critical path, and the ramped-up PE absorbs the whole
            # k-accumulation behind it.
            PE_PLANES = (4, 4, 3)
            # products per chunk offloaded to the gpsimd engine (0: the Q7
            # software tensor_tensor is far below DVE rate and contends for
            # SBUF ports with concurrent DVE passes)
            POOL_PRODS = (0, 0, 0)

            with (
                tc.tile_pool(name="prod", bufs=12) as pp,
                tc.tile_pool(name="ostage", bufs=2) as po,
                tc.tile_pool(name="mmpsum", bufs=2, space="PSUM") as pmm,
            ):
                for ch in range(NCH):
                    npe = PE_PLANES[ch]
                    tmps = []
                    kws = []
                    for v in range(S):
                        for wi, k in zip(wts[v], (1 + 3 * v, 2 + 3 * v, 3 + 3 * v)):
                            kws.append((wi, k))
                    # gpsimd-offloaded products first (emitted early so the
                    # pool engine starts while DVE streams its own products)
                    npool = POOL_PRODS[ch]
                    order = kws[:npool] + kws[npool:]
                    for i, (wi, k) in enumerate(order):
                        wv = (
                            wi[:]
                            .rearrange("p (d w2 pp) -> p d w2 pp", d=DQ, pp=2)[
                                :, ch * DCH : (ch + 1) * DCH, :, :
                            ]
                            .unsqueeze(3)
                            .to_broadcast([H, DCH, W2, G, 2])
                        )
                        tm = pp.tile([H, DCH * GW], f16, tag="tmp", name=f"tm{ch}")
                        tv_ = tm[:].rearrange(
                            "p (d w2 g pp) -> p d w2 g pp", d=DCH, g=G, pp=2
                        )
                        eng = nc.gpsimd if i < npool else nc.vector
                        eng.tensor_tensor(tv_, dotk(k), wv, Alu.mult)
                        tmps.append((tm, k))

                    # consume the gpsimd product(s) last so DVE/PE don't
                    # stall waiting for the slower pool engine
                    tmps_l = tmps[npool:] + tmps[:npool]

                    ost = po.tile([H, DCH * GW], f16, tag="ost", name=f"ost{ch}")
                    # --- PE-owned planes: PSUM accumulate + scalar drain ---
                    # TERM-MAJOR matmul emission within each plane: the base
                    # (DOT_0, available immediately) OPENS every region, each
                    # term streams as soon as its product lands, and the last
                    # product closes -- so only ~one term of matmuls remains
                    # after the final product.
                    for d in range(npe):
                        pt = pmm.tile([H, GW], f32, tag="accp")
                        for s0, s1 in segs:
                            nc.tensor.matmul(
                                pt[:, s0:s1],
                                ident_t[:],
                                dview[:, s0 // 16 : s1 // 16, 0, :, :],
                                start=True,
                                stop=False,
                            )
                        for i, (tm, k) in enumerate(tmps_l):
                            for s0, s1 in segs:
                                nc.tensor.matmul(
                                    pt[:, s0:s1],
                                    ident_t[:],
                                    tm[:, d * GW + s0 : d * GW + s1],
                                    start=False,
                                    stop=(i == len(tmps_l) - 1),
                                )
                        nc.scalar.activation(
                            ost[:, d * GW : (d + 1) * GW], pt[:], Act.Copy
                        )

                    # --- DVE-owned planes: fp16 in-place adds ---
                    if npe < DCH:
                        nd = DCH - npe
                        osl = ost[:, npe * GW : DCH * GW]
                        base = (
                            dview[:, :, 0, :, :]
                            .unsqueeze(1)
                            .to_broadcast([H, nd, W2, G, 2])
                        )
                        ov = osl.rearrange(
                            "p (d w2 g pp) -> p d w2 g pp", d=nd, g=G, pp=2
                        )
                        tm0, _ = tmps_l[0]
                        nc.vector.tensor_tensor(
                            ov, base, tm0[:, npe * GW :].rearrange(
                                "p (d w2 g pp) -> p d w2 g pp", d=nd, g=G, pp=2
                            ),
                            Alu.add,
                        )
                        for tm, k in tmps_l[1:]:
                            nc.vector.tensor_tensor(
                                osl, osl, tm[:, npe * GW :], Alu.add
                            )

                    nc.sync.dma_start(
                        out[ch * DCH : (ch + 1) * DCH, :, :].rearrange(
                            "d p c -> p d c"
                        ),
                        ost[:].rearrange("p (d c) -> p d c", d=DCH),
                    )

    nc.compile()
    _PROGRAM_CACHE["nc"] = nc
    return nc


def _host_prep(ref_feature, src_features, ref_proj, src_projs, depth_sample):
    """Projection-matrix chain bit-matched to the reference via jax CPU."""
    import jax
    import jax.numpy as jnp

    rot_xyz_all = np.zeros((S, B, 3, H, W), np.float32)
    trans_all = np.zeros((S, B, 3), np.float32)
    with jax.default_device(jax.devices("cpu")[0]):
        intr = jnp.asarray(INTR)
        intr_inv = jnp.asarray(INTR_INV)
        ref_p = intr_inv @ jnp.asarray(np.asarray(ref_proj))[:, :3, :4]  # [B,3,4]
        yy, xx = jnp.meshgrid(
            jnp.arange(H, dtype=jnp.float32), jnp.arange(W, dtype=jnp.float32),
            indexing="ij",
        )
        xyz = jnp.stack([xx.ravel(), yy.ravel(), jnp.ones(H * W, jnp.float32)])
        for s in range(S):
            src_p = intr_inv @ jnp.asarray(np.asarray(src_projs)[s])[:, :3, :4]
            proj = jnp.einsum("bij,bkj->bik", src_p[:, :, :3], ref_p[:, :, :3])
            trans = intr @ (src_p[:, :, 3:4] - proj @ ref_p[:, :, 3:4])
            rot = intr @ proj @ intr_inv
            rot_xyz = rot @ xyz  # [B,3,HW]
            rot_xyz_all[s] = np.asarray(rot_xyz).reshape(B, 3, H, W)
            trans_all[s] = np.asarray(trans).reshape(B, 3)

    # tap vectors: the 2x2 corner footprint of each (s,b) source image
    feats = np.asarray(src_features)
    tapv = np.zeros((S, B, 4, C), np.float32)
    for ti, (ty, tx) in enumerate(((0, 0), (0, 1), (1, 0), (1, 1))):
        tapv[:, :, ti, :] = feats[:, :, :, ty, tx]

    return rot_xyz_all, trans_all, tapv


def _check_degenerate(rot_xyz, trans, dep):
    """Verify, in a float32 mirror of the device computation, that for every
    pixel/plane/view: Z > 0.001 (zpos never fires), px,py < 1 (floor == 0 and
    the upper in-bounds masks never fire).  px,py >= 0 is NOT required (the
    device applies the >=0 mask via relu).  Conservative margins cover the
    device's fp16/approx-reciprocal differences."""
    for s in range(S):
        for b in range(B):
            rx = rot_xyz[s, b]
            t = trans[s, b]
            dq = dep[b]
            Z = rx[2] * dq + t[2]
            if Z.min() <= 0.0011:
                return False
            for k in (0, 1):
                P = (rx[k] * dq + t[k]) / Z
                if P.max() >= 0.995:
                    return False
    return True


def _fallback_numpy(rot_xyz, trans, refb, dep, src_features):
    """General (gather-based) host computation, used only if the degenerate
    fast-path assumption fails for the given inputs."""
    feats = np.asarray(src_features)
    P = np.ascontiguousarray(feats.transpose(0, 1, 3, 4, 2))  # [S,B,H,W,C]
    Px = np.roll(P, -1, axis=3)
    Py = np.roll(P, -1, axis=2)
    Pxy = np.roll(Py, -1, axis=3)
    tabs = np.concatenate([P, Px, Py, Pxy], axis=-1).reshape(S, B, HW, 4 * C)
    full = np.zeros((B, G, D, H, W), np.float32)
    for b in range(B):
        refb_b = refb[b].reshape(H, W, C)
        simacc = np.zeros((D, H, W, G), np.float32)
        for v in range(S):
            rx = rot_xyz[v, b][:, None]
            t = trans[v, b]
            dq = dep[b]
            X = rx[0] * dq + t[0]
            Y = rx[1] * dq + t[1]
            Z = rx[2] * dq + t[2]
            zm = (Z > 0.001).astype(np.float32)
            X, Y = X * zm, Y * zm
            Zc = np.where(Z > 0.001, Z, np.float32(1.0))
            px = X / Zc
            py = Y / Zc
            px = px * ((px < W) & (px >= 0)).astype(np.float32)
            py = py * ((py < H) & (py >= 0)).astype(np.float32)
            fx = px - np.floor(px)
            fy = py - np.floor(py)
            x0 = px - fx
            y0 = py - fy
            gx = np.float32(1.0) - fx
            gy = np.float32(1.0) - fy
            wts = [gx * gy, fx * gy, gx * fy, fx * fy]
            idx = (y0 * W + x0).astype(np.int32)
            gat = tabs[v, b][idx]
            R = (
                gat.reshape(D, H, W, 4, G, CPG)
                * refb_b.reshape(1, H, W, 1, G, CPG)
            ).sum(axis=-1)
            simacc += sum(R[:, :, :, ti, :] * wts[ti][..., None] for ti in range(4))
        full[b] = simacc.transpose(3, 0, 1, 2)
    return full


def _make_in_maps(ref_feature, src_features, ref_proj, src_projs, depth_sample):
    rot_xyz, trans, tapv = _host_prep(
        ref_feature, src_features, ref_proj, src_projs, depth_sample
    )
    dep = np.asarray(depth_sample)
    if not _check_degenerate(rot_xyz, trans, dep):
        refb = (
            np.asarray(ref_feature).transpose(0, 2, 3, 1) * np.float32(0.25)
        ).reshape(B, H, W * C)
        return None, (rot_xyz, trans, refb, dep)

    ref = np.asarray(ref_feature)  # [B,C,H,W]
    ident = np.eye(H, dtype=np.float16)

    # per-batch tensors
    reft_b = {}
    combos_b = {}
    rxh_b = {}
    tvec_b = {}
    for b in range(B):
        # reft[pp*32+c, w*64+h64] = ref[c, pp*64+h64, w]  (0.25 baked into combos)
        rt = (
            ref[b].reshape(C, 2, H2, W).transpose(1, 0, 3, 2).reshape(2 * C, PW)
        )
        reft_b[b] = rt.astype(np.float16)

        # combos (0.25-scaled), block-diag over pp and group-diag over g;
        # output-partition order q = (k*8+g)*2 + pp so the XBAR transpose
        # lands DOT in (w2, k, g, pp) free order:
        # combos[pp*32+c, (k*8+g)*2+pp] = 0.25*combo_k[c] if c//4==g
        A0, B0, C0, D0 = tapv[0, b]
        A1, B1, C1, D1 = tapv[1, b]
        ck = np.stack(
            [A0 + A1, B0 - A0, C0 - A0, A0 - B0 - C0 + D0,
             B1 - A1, C1 - A1, A1 - B1 - C1 + D1]
        ) * np.float32(0.25)  # [7, C]
        cb = np.zeros((2 * C, 2 * KG), np.float32)
        for k in range(7):
            for c in range(C):
                g = c // CPG
                for pps in range(2):
                    cb[pps * C + c, (k * G + g) * 2 + pps] = ck[k, c]
        combos_b[b] = cb.astype(np.float16)

        rx = rot_xyz[:, b]  # [S,3,H,W]
        # [p2=(wpar,h64), (j, w2, pp)]
        rxh_b[b] = (
            rx.reshape(S * 3, 2, H2, W // 2, 2)
            .transpose(4, 2, 0, 3, 1)
            .reshape(H, S * 3 * W)
        ).astype(np.float16)
        tv = np.zeros((H, 8), np.float32)
        tv[:, 0:3] = trans[0, b]
        tv[:, 3:6] = trans[1, b]
        tvec_b[b] = tv

    in_maps = []
    for kcore in range(NCORES):
        b, q = kcore // 4, kcore % 4
        dslc = dep[b, q * DQ : (q + 1) * DQ]  # [DQ,H,W]
        # [p2=(wpar,h64), (d, w2, pp)]
        dep_hp = (
            dslc.reshape(DQ, 2, H2, W // 2, 2)
            .transpose(4, 2, 0, 3, 1)
            .reshape(H, DQ * W)
            .astype(np.float16)
        )
        in_maps.append(
            {
                "reft": reft_b[b],
                "combos": combos_b[b],
                "ident": ident,
                "rxh": rxh_b[b],
                "tvec": tvec_b[b],
                "dep": np.ascontiguousarray(dep_hp),
            }
        )
    return in_maps, None


def kernel(ref_feature, src_features, ref_proj, src_projs, depth_sample):
    from concourse.bass_utils import run_bass_kernel_spmd

    in_maps, fb = _make_in_maps(
        ref_feature, src_features, ref_proj, src_projs, depth_sample
    )
    if in_maps is None:
        rot_xyz, trans, refb, dep = fb
        return _fallback_numpy(rot_xyz, trans, refb, dep, src_features)

    nc = _build_program()
    res = run_bass_kernel_spmd(nc, in_maps, core_ids=list(range(NCORES)))

    full = np.zeros((B, G, D, H, W), np.float32)
    for kcore in range(NCORES):
        b, q = kcore // 4, kcore % 4
        # out[d, p2=(wpar,h64), (w2, g, pp)] -> [g, d, h=(pp,h64), w=(w2,wpar)]
        o = res.results[kcore]["out"].astype(np.float32)
        o = o.reshape(DQ, 2, H2, W // 2, G, 2).transpose(4, 0, 5, 2, 3, 1)
        full[b, :, q * DQ : (q + 1) * DQ] = o.reshape(G, DQ, H, W)
    return full
